# revision 18
# baseline (speedup 1.0000x reference)
"""Trainium2 Bass kernel for nn_DecoderWithAttention (Show-Attend-Tell decoder).

Distribution: data-parallel over batch, 8 rows/core, one SPMD program on 8
NeuronCores, zero collectives.

Per core:
  Phase 0: load + fp32r-round weights, precompute eaT[a,(b,p)] = W_enc @ enc^T.
  Phase 1: 50 unrolled recurrence steps (additive attention + LSTM cell),
           storing masked h_t^T and masked alphas on-chip.
  Phase 2: one big fc matmul over all (t,b) rows x 32000 vocab, streaming
           W_fc^T chunks from DRAM.

Host: sorts by caption length, gathers embeddings, precomputes the embedding
half of the LSTM gate pre-activations, h0/c0, and reassembles outputs.

Matmuls run in float32r (full PE rate, ~1e-4 rel err). eaT / relu scores /
g_emb are bf16 to fit SBUF.
"""

import os
import sys
import types

import numpy as np
import ml_dtypes

import concourse.bacc as bacc
import concourse.mybir as mybir
import concourse.tile as tile
from concourse.bass_utils import run_bass_kernel_spmd
from concourse.masks import make_identity

F32 = mybir.dt.float32
F32R = mybir.dt.float32r
BF16 = mybir.dt.bfloat16
AF = mybir.ActivationFunctionType
ALU = mybir.AluOpType
AX = mybir.AxisListType

B, PS, ENC = 64, 14, 512
P = PS * PS  # 196
V, E, D, A = 32000, 512, 512, 512
G = 4 * D  # 2048
MAXLEN = 51
T = MAXLEN - 1  # 50
NCORE = 8
BL = B // NCORE  # 8
ROWS = T * BL  # 400
P2 = P - 128  # 68

TRACE = False
LAST_EXEC_NS = None

_CACHED = {}


def _install_ntff_hook():
    try:
        from antenv.axon_hooks import get_axon_ntff_profile_hook  # noqa: F401
        return
    except ImportError:
        pass
    mod = types.ModuleType("antenv.axon_hooks")
    _hook = [None]
    mod.set_axon_ntff_profile_hook = lambda h: _hook.__setitem__(0, h)
    mod.get_axon_ntff_profile_hook = lambda: _hook[0]
    sys.modules["antenv.axon_hooks"] = mod
    import antenv
    antenv.axon_hooks = mod
    try:
        from trn_agent_boot.trn_boot import _ntff_profile_via_ctypes
        mod.set_axon_ntff_profile_hook(
            _ntff_profile_via_ctypes("/opt/axon/libaxon_pjrt.so"))
    except Exception:
        pass


def pi(b):
    return 32 * (b // 2) + (b % 2)


# ctx matmul accumulation segments per batch row: (alphaT chunk c4, nrows,
# enc tile 'A'/'B'). alphaT chunk c = 2*(b%2) + (p >= 128).
def ctx_segments(b):
    jj = b % 2
    return [(2 * jj, 128, "A"), (2 * jj + 1, P - 128, "B")]


def build_program():
    nc = bacc.Bacc(trn_type="TRN2", dynamic_dma_scratch_size=4096)

    dram = {}

    def din(name, shape, dt=F32):
        dram[name] = nc.dram_tensor(name, list(shape), dt, kind="ExternalInput")
        return dram[name]

    def dout(name, shape, dt=F32):
        dram[name] = nc.dram_tensor(name, list(shape), dt, kind="ExternalOutput")
        return dram[name]

    din("encT", (128, 4, BL * P), F32R)
    din("encp", (BL, P, ENC), BF16)
    din("g_emb", (T, BL, G), BF16)
    din("h0T", (128, 4, BL), F32R)
    din("c0", (BL, D))
    din("mask_sc", (128, T))
    din("maskrow", (1, T, BL))
    din("W_encT", (128, 4, A), F32R)
    din("Wdg", (128, 4, A + ENC), F32R)
    din("Wcat", (128, 8, G), F32R)
    din("wdup", (128, 4, 2), BF16)
    din("bias_dg", (1, A + ENC))
    din("W_fcT", (128, 4, V), F32R)

    dout("preds", (ROWS, V))
    dout("alphas_sc", (128, 4, T, BL))

    with tile.TileContext(nc) as tc:
        with tc.tile_pool(name="sing", bufs=1) as sing, \
             tc.tile_pool(name="work", bufs=2) as work, \
             tc.tile_pool(name="psA", bufs=2, space="PSUM") as psA, \
             tc.tile_pool(name="psB", bufs=4, space="PSUM") as psB:

            # ---------------- identities ----------------
            id8 = sing.tile([8, 8], F32, tag="id8")
            make_identity(nc, id8)
            id98f = sing.tile([98, 98], F32, tag="id98f")
            make_identity(nc, id98f)

            # ---------------- load + round ----------------
            def load_round(name, shape):
                t_ = sing.tile(list(shape), F32R, tag=name)
                nc.sync.dma_start(out=t_, in_=dram[name][:])
                return t_

            Wdg = load_round("Wdg", (128, 4, A + ENC))
            Wcat = load_round("Wcat", (128, 8, G))

            encA = sing.tile([128, BL, ENC], BF16, tag="encA")
            encp_r = dram["encp"].rearrange("b p e -> p b e")
            nc.sync.dma_start(out=encA, in_=encp_r[0:128])
            encB = sing.tile([P2, BL, ENC], BF16, tag="encB")
            nc.sync.dma_start(out=encB, in_=encp_r[128:P])

            wdup = sing.tile([128, 4, 2], BF16, tag="wdup")
            nc.sync.dma_start(out=wdup, in_=dram["wdup"][:])
            bias_dg = sing.tile([8, A + ENC], F32, tag="bias_dg")
            nc.gpsimd.dma_start(
                out=bias_dg, in_=dram["bias_dg"][0:1].partition_broadcast(8))
            mask_sc = sing.tile([128, T], F32, tag="mask_sc")
            nc.sync.dma_start(out=mask_sc, in_=dram["mask_sc"][:])
            maskrow = sing.tile([128, T, BL], F32, tag="maskrow")
            nc.gpsimd.dma_start(
                out=maskrow, in_=dram["maskrow"][0:1].partition_broadcast(128))

            hT = sing.tile([128, 4, BL], F32R, tag="hT")
            nc.sync.dma_start(out=hT, in_=dram["h0T"][:])
            c_sb = sing.tile([BL, D], F32, tag="c_sb")
            nc.sync.dma_start(out=c_sb, in_=dram["c0"][:])

            Hstore = sing.tile([128, 4, ROWS], F32R, tag="Hstore")
            aacc = sing.tile([128, 4, T, BL], F32, tag="aacc")

            alpha_sc = sing.tile([98, 2 * P], F32, tag="alpha_sc")
            nc.vector.memset(alpha_sc, 0.0)
            sum_sb = sing.tile([98, 2], F32, tag="sum_sb")
            nc.vector.memset(sum_sb, 1.0)
            negmax = sing.tile([98, 2], F32, tag="negmax")
            nc.vector.memset(negmax, 0.0)
            recip = sing.tile([98, 2], F32, tag="recip")
            nc.vector.memset(recip, 1.0)
            ctx_sb = [sing.tile([98, ENC], F32, tag=f"ctx_sb{r}",
                                 name=f"ctx_sb{r}") for r in range(2)]
            nc.vector.memset(ctx_sb[0], 0.0)
            nc.vector.memset(ctx_sb[1], 0.0)

            # ---------------- eaT ----------------
            WencT = work.tile([128, 4, A], F32R, tag="wfc", name="WencT")
            nc.sync.dma_start(out=WencT, in_=dram["W_encT"][:])
            eaT = sing.tile([128, 4, BL, P], BF16, tag="eaT")
            NSZ = 392
            for pc in range(4):
                encT_p = work.tile([128, 4, NSZ], F32R, tag="wfc",
                                   name=f"encT{pc}")
                nc.sync.dma_start(
                    out=encT_p,
                    in_=dram["encT"][:, :, NSZ * pc:NSZ * (pc + 1)])
                for ac in range(4):
                    ea_flat = eaT[:, ac].rearrange("p b q -> p (b q)")
                    pst = psA.tile([128, 512], F32, tag="big")
                    for kc in range(4):
                        nc.tensor.matmul(
                            pst[:, :NSZ],
                            WencT[:, kc, 128 * ac:128 * (ac + 1)],
                            encT_p[:, kc, :],
                            start=(kc == 0), stop=(kc == 3))
                    nc.vector.tensor_copy(
                        out=ea_flat[:, NSZ * pc:NSZ * (pc + 1)],
                        in_=pst[:, :NSZ])

            # ---------------- recurrence ----------------
            for t in range(T):
                # (a) da | gate
                ps_dg = psA.tile([8, 2, 512], F32, tag="big")
                for half in range(2):
                    for kc in range(4):
                        nc.tensor.matmul(
                            ps_dg[:, half, :],
                            hT[:, kc, :],
                            Wdg[:, kc, 512 * half:512 * (half + 1)],
                            start=(kc == 0), stop=(kc == 3))
                dg_sb = work.tile([8, A + ENC], F32, tag="dg_sb", bufs=1)
                nc.vector.tensor_tensor(
                    out=dg_sb,
                    in0=ps_dg.rearrange("p a b -> p (a b)"),
                    in1=bias_dg,
                    op=ALU.add)
                gate_sb = work.tile([8, ENC], F32, tag="gate_sb", bufs=1)
                nc.scalar.activation(out=gate_sb, in_=dg_sb[:, A:],
                                     func=AF.Sigmoid)

                # (b) daT
                ps_daT = psB.tile([128, 4, 8], F32, tag="sm")
                for c4 in range(4):
                    nc.tensor.matmul(
                        ps_daT[:, c4, :],
                        dg_sb[:, 128 * c4:128 * (c4 + 1)], id8,
                        is_transpose=True, start=(c4 == 0), stop=(c4 == 3))
                daT = work.tile([128, 4, 8], F32, tag="daT", bufs=1)
                nc.vector.tensor_copy(out=daT, in_=ps_daT)

                # (c) relu(eaT + daT) -> bf16, split ACT/DVE
                r_sb = work.tile([128, 4, BL, P], BF16, tag="r_sb", bufs=1)
                for c4 in range(4):
                    for b in range(BL):
                        if c4 < 2:
                            nc.scalar.activation(
                                out=r_sb[:, c4, b, :], in_=eaT[:, c4, b, :],
                                func=AF.Relu, bias=daT[:, c4, b:b + 1])
                        else:
                            nc.vector.tensor_scalar(
                                out=r_sb[:, c4, b, :], in0=eaT[:, c4, b, :],
                                scalar1=daT[:, c4, b:b + 1], scalar2=0.0,
                                op0=ALU.add, op1=ALU.max)

                # (d) scores, col-tiled M=2
                ps_s = psB.tile([98, 2 * P], F32, tag="sm")
                for j in range(4):
                    rhs_j = r_sb[:, :, 2 * j:2 * j + 2, :]
                    for c4 in range(4):
                        nc.tensor.matmul(
                            ps_s[32 * j:32 * j + 2, :],
                            wdup[:, c4, :],
                            rhs_j[:, c4].rearrange("p b q -> p (b q)"),
                            start=(c4 == 0), stop=(c4 == 3),
                            tile_position=(0, 32 * j))

                # (e) softmax + mask
                for j in range(4):
                    nc.vector.tensor_reduce(
                        out=negmax[32 * j:32 * j + 2, :],
                        in_=ps_s[32 * j:32 * j + 2, :].rearrange(
                            "p (b q) -> p b q", b=2),
                        axis=AX.X, op=ALU.max, negate=True)
                for j in range(4):
                    for jj in range(2):
                        nc.scalar.activation(
                            out=alpha_sc[32 * j:32 * j + 2,
                                         P * jj:P * (jj + 1)],
                            in_=ps_s[32 * j:32 * j + 2, P * jj:P * (jj + 1)],
                            func=AF.Exp,
                            bias=negmax[32 * j:32 * j + 2, jj:jj + 1],
                            accum_out=sum_sb[32 * j:32 * j + 2, jj:jj + 1])
                nc.vector.reciprocal(out=recip, in_=sum_sb)
                for j in range(4):
                    for jj in range(2):
                        nc.vector.tensor_scalar(
                            out=alpha_sc[32 * j:32 * j + 2,
                                         P * jj:P * (jj + 1)],
                            in0=alpha_sc[32 * j:32 * j + 2,
                                         P * jj:P * (jj + 1)],
                            scalar1=recip[32 * j:32 * j + 2, jj:jj + 1],
                            scalar2=mask_sc[32 * j:32 * j + 2, t:t + 1],
                            op0=ALU.mult, op1=ALU.mult)

                # (f) alphaT
                ps_aT = psB.tile([128, 4, 128], F32, tag="sm")
                ACHUNKS = [(0, 128), (128, P - 128), (P, 128), (P + 128, P - 128)]
                for c4, (f0, w) in enumerate(ACHUNKS):
                    nc.tensor.matmul(
                        ps_aT[:w, c4, 0:98],
                        alpha_sc[:, f0:f0 + w], id98f,
                        is_transpose=True, start=(c4 == 0), stop=(c4 == 3))
                alphaT = work.tile([128, 4, 98], BF16, tag="alphaT", bufs=1)
                nc.vector.tensor_copy(out=alphaT, in_=ps_aT[:, :, 0:98])
                for b in range(BL):
                    nc.vector.tensor_copy(
                        out=aacc[:, :, t, b:b + 1],
                        in_=alphaT[:, :, pi(b):pi(b) + 1])

                # (g) ctx, per-b col-tiled, rounds by b%2
                ps_ctx = [psB.tile([98, ENC], F32, tag="sm",
                                   name=f"ps_ctx{t}_{r}") for r in range(2)]
                for b in range(BL):
                    j, r = b // 2, b % 2
                    segs = ctx_segments(b)
                    for si, (c4, nr, et) in enumerate(segs):
                        enc_t = encA if et == "A" else encB
                        nc.tensor.matmul(
                            ps_ctx[r][32 * j:32 * j + 1, :],
                            alphaT[0:nr, c4, 32 * j + r:32 * j + r + 1],
                            enc_t[0:nr, b, :],
                            start=(si == 0), stop=(si == len(segs) - 1),
                            tile_position=(0, 32 * j))
                for r in range(2):
                    for j in range(4):
                        nc.vector.tensor_copy(
                            out=ctx_sb[r][32 * j:32 * j + 1, :],
                            in_=ps_ctx[r][32 * j:32 * j + 1, :])

                # (h) gateT, ctxT, gctxT
                ps_gT = psB.tile([128, 4, 8], F32, tag="sm")
                for c4 in range(4):
                    nc.tensor.matmul(
                        ps_gT[:, c4, :],
                        gate_sb[:, 128 * c4:128 * (c4 + 1)], id8,
                        is_transpose=True, start=(c4 == 0), stop=(c4 == 3))
                ps_cT = [psB.tile([128, 4, 128], F32, tag="sm",
                                  name=f"ps_cT{t}_{r}") for r in range(2)]
                for r in range(2):
                    for c4 in range(4):
                        nc.tensor.matmul(
                            ps_cT[r][:, c4, 0:98],
                            ctx_sb[r][:, 128 * c4:128 * (c4 + 1)], id98f,
                            is_transpose=True, start=(c4 == 0), stop=(c4 == 3))
                gateT = work.tile([128, 4, BL], F32, tag="gateT", bufs=1)
                nc.vector.tensor_copy(out=gateT, in_=ps_gT)
                gctxT = work.tile([128, 4, BL], F32R, tag="gctxT", bufs=1)
                for r in range(2):
                    nc.vector.tensor_tensor(
                        out=gctxT[:, :, r:BL:2],
                        in0=ps_cT[r][:, :, 0:128:32],
                        in1=gateT[:, :, r:BL:2],
                        op=ALU.mult)

                # (i) LSTM gates (two halves of 1024 outputs)
                ge = work.tile([BL, G], BF16, tag="ge", bufs=3)
                nc.sync.dma_start(out=ge, in_=dram["g_emb"][t])
                gates = work.tile([8, G], F32, tag="gates", bufs=1)
                for half in range(2):
                    ps_g = psA.tile([8, 2, 512], F32, tag="big")
                    for nc_ in range(2):
                        col0 = 1024 * half + 512 * nc_
                        for kc in range(8):
                            lhs = gctxT if kc < 4 else hT
                            nc.tensor.matmul(
                                ps_g[:, nc_, :],
                                lhs[:, kc % 4, :],
                                Wcat[:, kc, col0:col0 + 512],
                                start=(kc == 0), stop=(kc == 7))
                    nc.vector.tensor_tensor(
                        out=gates[:, 1024 * half:1024 * (half + 1)],
                        in0=ps_g.rearrange("p a b -> p (a b)"),
                        in1=ge[:, 1024 * half:1024 * (half + 1)],
                        op=ALU.add)

                # (j) cell (activations in-place on gates)
                nc.scalar.activation(out=gates[:, 0:D], in_=gates[:, 0:D],
                                     func=AF.Sigmoid)
                nc.scalar.activation(out=gates[:, D:2 * D],
                                     in_=gates[:, D:2 * D], func=AF.Sigmoid)
                nc.scalar.activation(out=gates[:, 2 * D:3 * D],
                                     in_=gates[:, 2 * D:3 * D], func=AF.Tanh)
                nc.scalar.activation(out=gates[:, 3 * D:], in_=gates[:, 3 * D:],
                                     func=AF.Sigmoid)
                nc.vector.tensor_tensor(out=c_sb, in0=gates[:, D:2 * D],
                                        in1=c_sb, op=ALU.mult)
                ig = work.tile([8, D], F32, tag="ig", bufs=1)
                nc.vector.tensor_tensor(out=ig, in0=gates[:, 0:D],
                                        in1=gates[:, 2 * D:3 * D], op=ALU.mult)
                nc.vector.tensor_tensor(out=c_sb, in0=c_sb, in1=ig, op=ALU.add)
                tan_c = work.tile([8, D], F32, tag="tan_c", bufs=1)
                nc.scalar.activation(out=tan_c, in_=c_sb, func=AF.Tanh)
                h_new = work.tile([8, D], F32, tag="h_new", bufs=1)
                nc.vector.tensor_tensor(out=h_new, in0=gates[:, 3 * D:],
                                        in1=tan_c, op=ALU.mult)

                # (k) hT + masked H store
                ps_hT = psB.tile([128, 4, 8], F32, tag="sm")
                for c4 in range(4):
                    nc.tensor.matmul(
                        ps_hT[:, c4, :],
                        h_new[:, 128 * c4:128 * (c4 + 1)], id8,
                        is_transpose=True, start=(c4 == 0), stop=(c4 == 3))
                nc.vector.tensor_copy(out=hT, in_=ps_hT)
                nc.vector.tensor_tensor(
                    out=Hstore[:, :, BL * t:BL * (t + 1)],
                    in0=ps_hT,
                    in1=maskrow[:, t, :].unsqueeze(1).broadcast_to(
                        (128, 4, BL)),
                    op=ALU.mult)

            # ---------------- phase 2: fc ----------------
            VCS = [512] * 62 + [256]
            voff = 0
            for vci, vn in enumerate(VCS):
                wfc = work.tile([128, 4, 512], F32R, tag="wfc",
                                name=f"wfc{vci}")
                nc.sync.dma_start(
                    out=wfc[:, :, :vn],
                    in_=dram["W_fcT"][:, :, voff:voff + vn])
                for mc in range(4):
                    m = 128 if mc < 3 else ROWS - 384
                    ps_p = psA.tile([128, 512], F32, tag="big")
                    for kc in range(4):
                        nc.tensor.matmul(
                            ps_p[:m, :vn],
                            Hstore[:, kc, 128 * mc:128 * mc + m],
                            wfc[:, kc, :vn],
                            start=(kc == 0), stop=(kc == 3))
                    ob = work.tile([128, 512], F32, tag="ob")
                    nc.vector.tensor_copy(out=ob[:m, :vn], in_=ps_p[:m, :vn])
                    nc.sync.dma_start(
                        out=dram["preds"][128 * mc:128 * mc + m,
                                          voff:voff + vn],
                        in_=ob[:m, :vn])
                voff += vn

            nc.sync.dma_start(out=dram["alphas_sc"][:], in_=aacc)

    nc.compile()
    return nc


def _chunk_k(w, kchunks):
    """(K, N) -> [128, K//128, N] host layout for k-chunked rhs/lhsT."""
    kk, n = w.shape
    assert kk == 128 * kchunks
    return np.ascontiguousarray(
        w.reshape(kchunks, 128, n).transpose(1, 0, 2))


def kernel(**inputs):
    global LAST_EXEC_NS
    _install_ntff_hook()

    f32 = np.float32
    cap_len = np.asarray(inputs["cap_len"]).astype(np.int64)
    caps_in = np.asarray(inputs["encoded_captions"]).astype(np.int64)
    enc_in = np.asarray(inputs["encoder_out"]).astype(f32)
    emb_table = np.asarray(inputs["emb_table"]).astype(f32)
    W_enc = np.asarray(inputs["W_enc_attn"]).astype(f32)
    b_enc = np.asarray(inputs["b_enc_attn"]).astype(f32)
    W_dec = np.asarray(inputs["W_dec_attn"]).astype(f32)
    b_dec = np.asarray(inputs["b_dec_attn"]).astype(f32)
    w_full = np.asarray(inputs["w_full_attn"]).astype(f32)
    W_ih = np.asarray(inputs["W_ih"]).astype(f32)
    W_hh = np.asarray(inputs["W_hh"]).astype(f32)
    b_ih = np.asarray(inputs["b_ih"]).astype(f32)
    b_hh = np.asarray(inputs["b_hh"]).astype(f32)
    W_init_h = np.asarray(inputs["W_init_h"]).astype(f32)
    b_init_h = np.asarray(inputs["b_init_h"]).astype(f32)
    W_init_c = np.asarray(inputs["W_init_c"]).astype(f32)
    b_init_c = np.asarray(inputs["b_init_c"]).astype(f32)
    W_fbeta = np.asarray(inputs["W_f_beta"]).astype(f32)
    b_fbeta = np.asarray(inputs["b_f_beta"]).astype(f32)
    W_fc = np.asarray(inputs["W_fc"]).astype(f32)
    b_fc = np.asarray(inputs["b_fc"]).astype(f32)

    # ---- host prep ----
    order = np.argsort(-cap_len, kind="stable")
    cap_len_s = cap_len[order]
    dec_len = cap_len_s - 1
    caps = caps_in[order]
    enc = enc_in.reshape(B, P, ENC)[order]
    emb = emb_table[caps[:, :T]]                        # (B, T, E)
    g_emb = emb @ W_ih[:, :E].T + (b_ih + b_hh)         # (B, T, G)
    mean_enc = enc.mean(axis=1)
    h0 = mean_enc @ W_init_h.T + b_init_h
    c0 = mean_enc @ W_init_c.T + b_init_c
    mask = (np.arange(T)[None, :] < dec_len[:, None]).astype(f32)  # (B, T)

    # shared weights
    Wdg_h = _chunk_k(np.concatenate([W_dec.T, W_fbeta.T], axis=1), 4)
    Wcat_h = _chunk_k(np.concatenate([W_ih[:, E:].T, W_hh.T], axis=0), 8)
    WencT_h = _chunk_k(np.ascontiguousarray(W_enc.T), 4)
    WfcT_h = _chunk_k(np.ascontiguousarray(W_fc.T), 4)
    wdup_h = _chunk_k(np.stack([w_full, w_full], axis=1), 4).astype(
        ml_dtypes.bfloat16)
    bias_dg_h = np.concatenate([b_dec + b_enc, b_fbeta])[None, :].astype(f32)

    shared = dict(W_encT=WencT_h, Wdg=Wdg_h, Wcat=Wcat_h, wdup=wdup_h,
                  bias_dg=np.ascontiguousarray(bias_dg_h), W_fcT=WfcT_h)

    in_maps = []
    for c in range(NCORE):
        rs = slice(BL * c, BL * (c + 1))
        enc_c = enc[rs]                                  # (8, 196, 512)
        encT_c = _chunk_k(
            np.ascontiguousarray(enc_c.transpose(2, 0, 1).reshape(ENC, BL * P)),
            4)
        h0T_c = _chunk_k(np.ascontiguousarray(h0[rs].T), 4)
        gemb_c = np.ascontiguousarray(g_emb[rs].transpose(1, 0, 2))  # (T, 8, G)
        mask_c = mask[rs]                                # (8, T)
        mask_sc_c = np.zeros((128, T), dtype=f32)
        for b in range(BL):
            mask_sc_c[pi(b), :] = mask_c[b]
        maskrow_c = np.ascontiguousarray(
            mask_c.T[None, :, :])                        # (1, T, 8)
        in_maps.append(dict(
            encT=encT_c,
            encp=np.ascontiguousarray(enc_c).astype(ml_dtypes.bfloat16),
            g_emb=gemb_c.astype(ml_dtypes.bfloat16),
            h0T=h0T_c,
            c0=np.ascontiguousarray(c0[rs]),
            mask_sc=mask_sc_c,
            maskrow=maskrow_c,
            **shared))

    # ---- build / run ----
    if "nc" not in _CACHED:
        _CACHED["nc"] = build_program()
    nc = _CACHED["nc"]

    res = run_bass_kernel_spmd(nc, in_maps, core_ids=list(range(NCORE)),
                               trace=TRACE)
    LAST_EXEC_NS = res.exec_time_ns

    # ---- assemble outputs ----
    predictions = np.zeros((B, T, V), dtype=f32)
    alphas = np.zeros((B, T, P), dtype=f32)
    pidx = np.arange(P)
    for c in range(NCORE):
        pr = res.results[c]["preds"].reshape(T, BL, V)
        predictions[BL * c:BL * (c + 1)] = pr.transpose(1, 0, 2)
        aacc = res.results[c]["alphas_sc"].astype(np.float32)               # (128, 4, T, 8)
        for b in range(BL):
            cc = 2 * (b % 2) + (pidx >= 128)
            rr = pidx % 128
            alphas[BL * c + b] = aacc[rr, cc, :, b].T
    if np.any(b_fc != 0):
        predictions += (b_fc[None, None, :] * mask[:, :, None])

    return (predictions, caps.astype(np.int32), dec_len.astype(np.int32),
            alphas, order.astype(np.int32))
